# revision 23
# baseline (speedup 1.0000x reference)
"""Trainium2 Bass kernel for Luong local-p sparse attention.

Math (per batch n, full shapes N=64, L=258, H=1024, Q=256):
    score = (h_t @ W_a) @ enc^T           masked to window [p_t-16, p_t+16]
    align = softmax(score) * gauss(p_t)
    out   = tanh([align @ enc, h_t] @ W_c^T)

Only a 33-wide window of enc survives the mask, so windows are gathered
host-side and W_a / W_c[:, :H] are pushed through the 33-wide side:
    u  = W_a-transform of window   (uT[h, (n,j)]  = sum_k W_aT[k,h] enc_w[(n,j),k])
    s  = uT^T-partial scores       (score^T[j, q] = sum_h uT[h,j] h_t[q,h])
    softmax over j (33 rows) done j-major with a 4th-power renormalization
    trick (no partition-dim max needed; partition sums via ones-matmul)
    v  = W_c1-transform of window  (v[(n,j), h']  = sum_h enc_w[(n,j),h] W_c1T[h,h'])
    out = tanh(h_t @ W_c2T + align-stack @ v)

Dtype strategy (measured rates): f32r streams 1 row / 2.4GHz cycle
(fp32_mode=HIGH) but its 128-col LDWEIGHTS costs 187ns; fp16 streams at
2.0GHz with 116ns LDWEIGHTS.  Long streams (v, W_c2, ctx, score) run
f32r; the u phase (M=264 with a fresh weight block per matmul,
LDWEIGHTS-bound) runs fp16 directly off the wire.  Everything heavy
travels the wire as fp16 (same 11-bit mantissa the PE keeps in f32r
mode) and is upcast on-chip only where f32r compute needs it.  Engine
emission order follows data-arrival order — each engine queue is
in-order, so a cast emitted too early blocks everything behind it.

ctx contracts over a whole 99-row v group with a zero-padded align
stack (t3): batch n's 33 softmax rows sit at partition (n%3)*W, zero
rows cancel the other windows, so the v output never needs
partition-shuffle DMAs and stays matmul-aligned.  The last batch
interleaves ctx/tanh/store per W_c2 quarter to cut the drain tail.

Data parallel over batch: 8 batches per core x 8 cores.  Output is
stored fp16 (tanh output in [-1,1]; host upcasts).
"""

import numpy as np

import concourse.bass as bass
import concourse.bacc as bacc
import concourse.mybir as mybir
import concourse.tile as tile
from concourse.bass_utils import run_bass_kernel_spmd

# Problem constants (hardcoded per harness contract).
N, L, H, Q = 64, 258, 1024, 256
WINDOW = 16.0
DEV_POW = 128.0
NCORES = 8
B = N // NCORES  # batches per core
W = 33           # window width (positions that can survive the mask)
HC = H // 128    # h-chunks of 128 (PE contraction tiles)
PK = H + B * W   # packed [W_aT | enc_wT] row width
F32 = mybir.dt.float32
F32R = mybir.dt.float32r
F16 = mybir.dt.float16
AF = mybir.ActivationFunctionType

# exp is computed as t = exp(s/4 + bias); bias = LOG_ALPHA keeps the
# column-sum T = sum_j t below fp32 max.  alpha cancels in w = t/T.
LOG_ALPHA = -4.8520302  # -7*ln(2)
MASK_BIAS = -10000.0    # exp(<= -9900) == 0 in fp32

# v-phase window groups: batches gi*3+off live at partitions off*W..off*W+32
GROUPS = [(0, 99), (99, 99), (198, 66)]


def build_nc() -> bass.Bass:
    nc = bacc.Bacc()
    pk16 = nc.declare_dram_parameter("pk16", [H, PK], F16, isOutput=False)
    dec_hT = nc.declare_dram_parameter("dec_hT", [H, B * Q], F16, isOutput=False)
    W_c1T = nc.declare_dram_parameter("W_c1T", [H, H], F16, isOutput=False)
    W_c2T = nc.declare_dram_parameter("W_c2T", [H, H], F16, isOutput=False)
    constsD = nc.declare_dram_parameter("constsD", [3 * W, B + 3], F32, isOutput=False)
    onesD = nc.declare_dram_parameter("onesD", [W, W], F32R, isOutput=False)
    zerosD = nc.declare_dram_parameter("zerosD", [3 * W, Q], F32R, isOutput=False)
    out = nc.declare_dram_parameter("out", [B * Q, H], F16, isOutput=True)

    with tile.TileContext(nc) as tc:
        with (
            tc.tile_pool(name="const", bufs=1) as cpool,
            tc.tile_pool(name="wstg", bufs=3) as wstg_pool,
            tc.tile_pool(name="wc1p", bufs=8) as wc1p,
            tc.tile_pool(name="vstp", bufs=6) as vstp,
            tc.tile_pool(name="dec16p", bufs=2) as dec16p,
            tc.tile_pool(name="dec", bufs=3) as dec_pool,
            tc.tile_pool(name="sm", bufs=4) as sm_pool,
            tc.tile_pool(name="outp", bufs=2) as out_pool,
            tc.tile_pool(name="psA", bufs=2, space="PSUM") as psA,
            tc.tile_pool(name="psB", bufs=6, space="PSUM") as psB,
        ):
            # ---------------- resident tensors ----------------
            pk_sb = cpool.tile([128, HC, PK], F16)    # [W_aT | enc] fp16
            enc32 = cpool.tile([128, HC, B * W], F32R)
            Wc2T_sb = cpool.tile([128, HC, H], F32R)
            consts_sb = cpool.tile([3 * W, B + 3], F32)
            ones_sb = cpool.tile([W, W], F32R)
            uT_sb = cpool.tile([128, HC, B * W], F32R)
            # zero-padded align stacks: ctx contracts over a whole v group;
            # batch n's 33 softmax rows sit at partition (n%3)*W and the
            # zero rows cancel the other batches' windows exactly.
            t3_sb = [cpool.tile([3 * W, Q], F32R, name=f"t3_{o}") for o in range(3)]
            bias_ap = consts_sb[0:W, 0:B]

            pk_r = pk16[:, :].rearrange("(c p) m -> p c m", p=128)
            Wc2_r = W_c2T[:, :].rearrange("(c p) m -> p c m", p=128)
            Wc1_r = W_c1T[:, :].rearrange("(cp p) m -> p cp m", p=128)
            dec_r = dec_hT[:, :].rearrange("(c p) (n q) -> p c n q", p=128, q=Q)

            # ---------------- DMA kickoff (issue order == priority) -------
            # chunk 0 lands in three pieces so the very first u matmul
            # (W_aT cols 0:128 + enc cols) can start ~2us earlier
            nc.sync.dma_start(out=pk_sb[:, 0, 0:128], in_=pk_r[:, 0, 0:128])
            nc.gpsimd.dma_start(out=pk_sb[:, 0, H:PK], in_=pk_r[:, 0, H:PK])
            nc.scalar.dma_start(out=pk_sb[:, 0, 128:H], in_=pk_r[:, 0, 128:H])
            qrr = [nc.sync, nc.gpsimd, nc.scalar]
            for kc in range(1, HC):
                qrr[kc % 3].dma_start(out=pk_sb[:, kc, :], in_=pk_r[:, kc, :])
            nc.gpsimd.dma_start(out=consts_sb, in_=constsD[:, :])
            nc.sync.dma_start(out=ones_sb, in_=onesD[:, :])
            for o in range(3):
                qrr[o].dma_start(out=t3_sb[o][:, :], in_=zerosD[:, :])

            # dec batches 0-1 early on scalar queue (needed at first score)
            dec16_tiles = {}
            for n in range(2):
                d16 = dec16p.tile([128, HC, Q], F16, tag="d16", name=f"d16_{n}")
                nc.scalar.dma_start(out=d16, in_=dec_r[:, :, n, :])
                dec16_tiles[n] = d16

            # W_c1 staged fp16 (v phase ~19us), then W_c2 quarters (~28us);
            # upcasts are emitted later, in consumption order.
            wc1_stage = {}
            for nt in range(2):
                for kcp in range(4):
                    st = wstg_pool.tile([128, 2, 512], F16, tag="w16",
                                        name=f"wc1s_{nt}_{kcp}")
                    eng = nc.sync if kcp % 2 == 0 else nc.gpsimd
                    eng.dma_start(
                        out=st,
                        in_=Wc1_r[:, 2 * kcp:2 * kcp + 2, nt * 512:(nt + 1) * 512],
                    )
                    wc1_stage[(nt, kcp)] = st
            wc2_stage = {}
            for qtr in range(4):
                st = wstg_pool.tile([128, HC, 256], F16, tag="w16", name=f"wc2s_{qtr}")
                eng = nc.sync if qtr % 2 == 0 else nc.gpsimd
                eng.dma_start(out=st, in_=Wc2_r[:, :, qtr * 256:(qtr + 1) * 256])
                wc2_stage[qtr] = st

            # ---------------- u phase: uT[h, (n,j)], kc-outer, fp16 -------
            # fp16 matmuls: 116ns LDWEIGHTS under a 134ns stream beats
            # f32r's 187ns LDWEIGHTS over a 117ns stream.
            pu = {}
            for kc in range(HC):
                for hc in range(HC):
                    if kc == 0:
                        pool = psB if hc < 6 else psA
                        tag = "B" if hc < 6 else "A"
                        pu[hc] = pool.tile([128, B * W], F32, tag=tag, name=f"pu{hc}")
                    nc.tensor.matmul(
                        pu[hc],
                        lhsT=pk_sb[:, kc, hc * 128:(hc + 1) * 128],
                        rhs=pk_sb[:, kc, H:],
                        start=(kc == 0),
                        stop=(kc == HC - 1),
                    )
            for hc in range(HC):
                if hc % 2 == 0:
                    nc.vector.tensor_copy(out=uT_sb[:, hc, :], in_=pu[hc])
                else:
                    nc.scalar.copy(out=uT_sb[:, hc, :], in_=pu[hc])

            # enc upcast for the v phase (f32r lhsT), in arrival order
            for kc in range(HC):
                nc.vector.tensor_copy(out=enc32[:, kc, :], in_=pk_sb[:, kc, H:])

            # ---------------- v phase: v[(n,j), h'] = enc_w @ W_c1T -------
            # vst tiles stay resident; ctx contracts over the whole group.
            wc1_tiles = {}
            for nt in range(2):
                for kcp in range(4):
                    wt = wc1p.tile([128, 2, 512], F32R, tag="wc1",
                                   name=f"wc1_{nt}_{kcp}")
                    if kcp % 2 == 0:
                        nc.scalar.copy(out=wt, in_=wc1_stage[(nt, kcp)])
                    else:
                        nc.vector.tensor_copy(out=wt, in_=wc1_stage[(nt, kcp)])
                    wc1_tiles[(nt, kcp)] = wt
            vst_tiles = {}
            for nt in range(2):
                for gi in range(3):
                    g0, glen = GROUPS[gi]
                    pv = psB.tile([128, 512], F32, tag="B", name=f"pv{nt}_{gi}")
                    for kcp in range(4):
                        for j in range(2):
                            kc = 2 * kcp + j
                            nc.tensor.matmul(
                                pv[:glen, :],
                                lhsT=enc32[:, kc, g0:g0 + glen],
                                rhs=wc1_tiles[(nt, kcp)][:, j, :],
                                start=(kc == 0),
                                stop=(kc == HC - 1),
                            )
                    vst = vstp.tile([128, 512], F32R, tag="vst", name=f"vst{nt}_{gi}")
                    # evacuate + fold the gaussian in one op
                    gp = consts_sb[0:glen, B + gi:B + gi + 1]
                    if (nt + gi) % 2 == 0:
                        nc.vector.tensor_scalar_mul(vst[:glen, :], pv[:glen, :], gp)
                    else:
                        nc.scalar.activation(
                            out=vst[:glen, :], in_=pv[:glen, :], func=AF.Copy, scale=gp
                        )
                    vst_tiles[(nt, gi)] = vst

            # W_c2 upcasts (consumed from ~29us)
            for qtr in range(4):
                dst = Wc2T_sb[:, :, qtr * 256:(qtr + 1) * 256]
                if qtr % 2 == 0:
                    nc.scalar.copy(out=dst, in_=wc2_stage[qtr])
                else:
                    nc.vector.tensor_copy(out=dst, in_=wc2_stage[qtr])

            # ---------------- per-batch pipeline ----------------
            prev = None  # (n, pos, o_sb) awaiting tanh + store

            def flush_prev():
                nonlocal prev
                if prev is None:
                    return
                pn, ppos, po_sb = prev
                for qt in range(2):
                    for ht in range(2):
                        nc.scalar.activation(
                            out=po_sb[:, qt, ht * 512:(ht + 1) * 512],
                            in_=ppos[(qt, ht)], func=AF.Tanh,
                        )
                dst = out[pn * Q:(pn + 1) * Q, :].rearrange("(qt p) h -> p qt h", p=128)
                eng = nc.sync if pn % 2 == 0 else nc.gpsimd
                eng.dma_start(out=dst, in_=po_sb)
                prev = None

            state = {}
            scored = {}

            def score_part(n):
                d16 = dec16_tiles.pop(n)
                # prefetch dec two batches ahead on the scalar queue
                if n + 2 < B and n + 2 not in dec16_tiles:
                    nxt = dec16p.tile([128, HC, Q], F16, tag="d16", name=f"d16_{n+2}")
                    nc.scalar.dma_start(out=nxt, in_=dec_r[:, :, n + 2, :])
                    dec16_tiles[n + 2] = nxt
                dec_sb = dec_pool.tile([128, HC, Q], F32R, tag="dec", name=f"dec{n}")
                nc.vector.tensor_copy(out=dec_sb, in_=d16)

                ps = psA.tile([W, Q], F32, tag="A", name=f"ps{n}")
                for hc in range(HC):
                    nc.tensor.matmul(
                        ps,
                        lhsT=uT_sb[:, hc, n * W:(n + 1) * W],
                        rhs=dec_sb[:, hc, :],
                        start=(hc == 0),
                        stop=(hc == HC - 1),
                    )
                # softmax over 33 partitions via 4th-power renormalization:
                # t=exp(s/4+b); T=colsum t; t=(t/T)^4; Z=colsum t; t/=Z.
                t = sm_pool.tile([W, Q], F32R, tag="t", name=f"t{n}")
                nc.scalar.activation(
                    out=t, in_=ps, func=AF.Exp, bias=bias_ap[:, n:n + 1], scale=0.25
                )
                scored[n] = (dec_sb, t)

            def softmax_T(n, t):
                pT = psA.tile([W, Q], F32, tag="A", name=f"pT{n}")
                nc.tensor.matmul(pT, lhsT=ones_sb[:], rhs=t[:], start=True, stop=True)
                rT = sm_pool.tile([W, Q], F32, tag="rT", name=f"rT{n}")
                nc.vector.reciprocal_approx_fast(out=rT, in_=pT)
                nc.vector.tensor_mul(t, t, rT)
                nc.vector.tensor_mul(t, t, t)
                nc.vector.tensor_mul(t, t, t)

            def softmax_Z(n, t):
                pZ = psA.tile([W, Q], F32, tag="A", name=f"pZ{n}")
                nc.tensor.matmul(pZ, lhsT=ones_sb[:], rhs=t[:], start=True, stop=True)
                rZ = sm_pool.tile([W, Q], F32, tag="rZ", name=f"rZ{n}")
                nc.vector.reciprocal_approx_fast(out=rZ, in_=pZ)
                tf = sm_pool.tile([W, Q], F32R, tag="tf", name=f"tf{n}")
                nc.vector.tensor_mul(tf, t, rZ)
                off = n % 3
                eng = nc.sync if n % 2 == 0 else nc.gpsimd
                eng.dma_start(out=t3_sb[off][off * W:(off + 1) * W, :], in_=tf[:, :])

            def dec_group(n, dec_sb, pos, qt, ht, start=True, stop=False):
                if start:
                    po = psB.tile([128, 512], F32, tag="B", name=f"po{n}_{qt}_{ht}")
                    pos[(qt, ht)] = po
                else:
                    po = pos[(qt, ht)]
                for hc in range(HC):
                    nc.tensor.matmul(
                        po,
                        lhsT=dec_sb[:, hc, qt * 128:(qt + 1) * 128],
                        rhs=Wc2T_sb[:, hc, ht * 512:(ht + 1) * 512],
                        start=start and (hc == 0),
                        stop=stop and (hc == HC - 1),
                    )

            def ctx_mm(n, pos, qt, ht, stop=True):
                gi, off = divmod(n, 3)
                glen = GROUPS[gi][1]
                nc.tensor.matmul(
                    pos[(qt, ht)],
                    lhsT=t3_sb[off][0:glen, qt * 128:(qt + 1) * 128],
                    rhs=vst_tiles[(ht, gi)][0:glen, :],
                    start=False,
                    stop=stop,
                )

            def batch_pre(n):
                if n not in scored:
                    score_part(n)
                dec_sb, t = scored.pop(n)
                flush_prev()
                o_sb = out_pool.tile([128, 2, H], F16, tag="o", name=f"o{n}")
                pos = {}
                dec_group(n, dec_sb, pos, 0, 0)
                softmax_T(n, t)
                dec_group(n, dec_sb, pos, 0, 1)
                dec_group(n, dec_sb, pos, 1, 0)
                softmax_Z(n, t)
                dec_group(n, dec_sb, pos, 1, 1)
                state[n] = (pos, o_sb)

            def batch_ctx(n):
                pos, o_sb = state.pop(n)
                for qt in range(2):
                    for ht in range(2):
                        ctx_mm(n, pos, qt, ht)
                nonlocal prev
                prev = (n, pos, o_sb)

            def batch_last(n):
                # softmax for n already ran during batch n-1; each W_c2
                # quarter closes with its ctx and drains tanh+store while
                # the next quarter streams.
                dec_sb, _ = scored.pop(n)
                flush_prev()
                o_sb = out_pool.tile([128, 2, H], F16, tag="o", name=f"o{n}")
                dst = out[n * Q:(n + 1) * Q, :].rearrange("(qt p) h -> p qt h", p=128)
                pos = {}

                def drain(qt, ht):
                    ctx_mm(n, pos, qt, ht)
                    nc.scalar.activation(
                        out=o_sb[:, qt, ht * 512:(ht + 1) * 512],
                        in_=pos[(qt, ht)], func=AF.Tanh,
                    )
                    eng = nc.sync if (qt + ht) % 2 == 0 else nc.gpsimd
                    eng.dma_start(
                        out=dst[:, qt, ht * 512:(ht + 1) * 512],
                        in_=o_sb[:, qt, ht * 512:(ht + 1) * 512],
                    )

                dec_group(n, dec_sb, pos, 0, 0)
                dec_group(n, dec_sb, pos, 0, 1)
                drain(0, 0)
                drain(0, 1)
                dec_group(n, dec_sb, pos, 1, 0)
                drain(1, 0)
                dec_group(n, dec_sb, pos, 1, 1)
                drain(1, 1)

            for n in range(B - 1):
                batch_pre(n)
                if n == B - 2:
                    # emit the last batch's score+softmax here so its t3 is
                    # ready before batch_last's interleaved ctx/drain
                    score_part(B - 1)
                    _, t_last = scored[B - 1]
                    softmax_T(B - 1, t_last)
                    softmax_Z(B - 1, t_last)
                batch_ctx(n)
            batch_last(B - 1)
    nc.compile()
    return nc


def prepare_in_maps(inputs: dict) -> list[dict]:
    enc = np.asarray(inputs["encoder_outputs"], dtype=np.float32)
    dec = np.asarray(inputs["decoder_h_t"], dtype=np.float32)
    src_len = np.asarray(inputs["src_len"], dtype=np.int32)
    p_t = np.asarray(inputs["p_t"], dtype=np.float32)
    W_a = np.asarray(inputs["W_a"], dtype=np.float32)
    W_c = np.asarray(inputs["W_c"], dtype=np.float32)

    # Window bounds, computed with the same fp32 ops as the reference.
    attn_start = np.maximum(p_t - np.float32(WINDOW), np.float32(0.0))
    attn_end = np.minimum(p_t + np.float32(WINDOW), src_len.astype(np.float32))
    s = np.ceil(attn_start).astype(np.int64)
    s = np.minimum(s, L - W)  # keep the 33-slice in bounds
    idx = s[:, None] + np.arange(W)[None, :]
    idxf = idx.astype(np.float32)
    mask = (idxf < attn_start[:, None]) | (idxf > attn_end[:, None])
    bias = np.where(mask, np.float32(MASK_BIAS), np.float32(LOG_ALPHA)).astype(np.float32)
    g = np.exp(-((idxf - p_t[:, None]) ** 2) / np.float32(DEV_POW)).astype(np.float32)

    enc_w = enc[np.arange(N)[:, None], idx, :].astype(np.float16)  # [N, W, H]
    dec = dec.astype(np.float16)
    W_aT16 = W_a.T.astype(np.float16)
    W_c1T = W_c[:, :H].T.astype(np.float16)
    W_c2T = W_c[:, H:].T.astype(np.float16)

    in_maps = []
    for c in range(NCORES):
        bs = slice(c * B, (c + 1) * B)
        enc_wT = enc_w[bs].transpose(2, 0, 1).reshape(H, B * W)  # [H, B*W]
        pk = np.concatenate([W_aT16, enc_wT], axis=1)            # [H, H+B*W]
        gc = g[bs]  # [B, W]
        consts = np.zeros((3 * W, B + 3), dtype=np.float32)
        consts[0:W, 0:B] = bias[bs].T
        for n in range(B):
            gi, off = divmod(n, 3)
            consts[off * W:(off + 1) * W, B + gi] = gc[n]
        in_maps.append({
            "pk16": np.ascontiguousarray(pk),
            "dec_hT": np.ascontiguousarray(dec[bs].transpose(2, 0, 1).reshape(H, B * Q)),
            "W_c1T": W_c1T,
            "W_c2T": W_c2T,
            "constsD": consts,
            "onesD": np.ones((W, W), dtype=np.float32),
            "zerosD": np.zeros((3 * W, Q), dtype=np.float32),
        })
    return in_maps


_NC = None


def get_nc() -> bass.Bass:
    global _NC
    if _NC is None:
        _NC = build_nc()
    return _NC


def kernel(**inputs) -> np.ndarray:
    nc = get_nc()
    in_maps = prepare_in_maps(inputs)
    res = run_bass_kernel_spmd(nc, in_maps, list(range(NCORES)))
    outs = [np.asarray(res.results[c]["out"], dtype=np.float32).reshape(B, Q, H)
            for c in range(NCORES)]
    return np.concatenate(outs, axis=0)


# revision 24
# speedup vs baseline: 1.1878x; 1.1878x over previous
"""Trainium2 Bass kernel for Luong local-p sparse attention.

Math (per batch n, full shapes N=64, L=258, H=1024, Q=256):
    score = (h_t @ W_a) @ enc^T           masked to window [p_t-16, p_t+16]
    align = softmax(score) * gauss(p_t)
    out   = tanh([align @ enc, h_t] @ W_c^T)

Only a 33-wide window of enc survives the mask, so windows are gathered
host-side and W_a / W_c[:, :H] are pushed through the 33-wide side:
    u  = W_a-transform of window   (uT[h, (n,j)]  = sum_k W_aT[k,h] enc_w[(n,j),k])
    s  = uT^T-partial scores       (score^T[j, q] = sum_h uT[h,j] h_t[q,h])
    softmax over j (33 rows) done j-major with a 4th-power renormalization
    trick (no partition-dim max needed; partition sums via ones-matmul)
    v  = W_c1-transform of window  (v[(n,j), h']  = sum_h enc_w[(n,j),h] W_c1T[h,h'])
    out = tanh(h_t @ W_c2T + align-stack @ v)

All matmuls run float32r (fp32_mode=HIGH streams 1 row / 2.4GHz cycle;
fp16/bf16 only stream at 2.0GHz, measured).  Everything heavy travels
the wire as fp16 (the same 11-bit mantissa the PE keeps in f32r mode)
and is upcast on-chip, halving load DMA.  Per-engine instruction queues
are strictly in-order, so every cast/exp/DMA-issue is emitted in the
order its inputs arrive — an early-emitted late-input op would block
the whole queue (measured 3-5us priority-inversion stalls otherwise).

The u phase runs kc-outer across all 8 PSUM banks so the PE starts on
the first W_aT chunk; the v phase follows while W_c2 still streams in.
ctx contracts over a whole 99-row v group with a zero-padded align
stack (t3): batch n's 33 softmax rows sit at partition (n%3)*W, zero
rows cancel the other windows, so the v output never needs
partition-shuffle DMAs and stays matmul-aligned (matmul operands must
start at partition 0/32/64).  The last batch's softmax runs during
batch 6 and its ctx/tanh/store drain per W_c2 quarter to cut the tail.

Data parallel over batch: 8 batches per core x 8 cores.  Output is
stored fp16 (tanh output in [-1,1]; host upcasts).
"""

import numpy as np

import concourse.bass as bass
import concourse.bacc as bacc
import concourse.mybir as mybir
import concourse.tile as tile
from concourse.bass_utils import run_bass_kernel_spmd

# Problem constants (hardcoded per harness contract).
N, L, H, Q = 64, 258, 1024, 256
WINDOW = 16.0
DEV_POW = 128.0
NCORES = 8
B = N // NCORES  # batches per core
W = 33           # window width (positions that can survive the mask)
HC = H // 128    # h-chunks of 128 (PE contraction tiles)
PK = H + B * W   # packed [W_aT | enc_wT] row width
F32 = mybir.dt.float32
F32R = mybir.dt.float32r
F16 = mybir.dt.float16
AF = mybir.ActivationFunctionType

# exp is computed as t = exp(s/4 + bias); bias = LOG_ALPHA keeps the
# column-sum T = sum_j t below fp32 max.  alpha cancels in w = t/T.
LOG_ALPHA = -4.8520302  # -7*ln(2)
MASK_BIAS = -10000.0    # exp(<= -9900) == 0 in fp32

# v-phase window groups: batches gi*3+off live at partitions off*W..off*W+32
GROUPS = [(0, 99), (99, 99), (198, 66)]


def build_nc() -> bass.Bass:
    nc = bacc.Bacc()
    pk16 = nc.declare_dram_parameter("pk16", [H, PK], F16, isOutput=False)
    dec_hT = nc.declare_dram_parameter("dec_hT", [H, B * Q], F16, isOutput=False)
    W_c1T = nc.declare_dram_parameter("W_c1T", [H, H], F16, isOutput=False)
    W_c2T = nc.declare_dram_parameter("W_c2T", [H, H], F16, isOutput=False)
    constsD = nc.declare_dram_parameter("constsD", [3 * W, B + 3], F32, isOutput=False)
    onesD = nc.declare_dram_parameter("onesD", [W, W], F32R, isOutput=False)
    zerosD = nc.declare_dram_parameter("zerosD", [3 * W, 3 * Q], F32R, isOutput=False)
    out = nc.declare_dram_parameter("out", [B * Q, H], F16, isOutput=True)

    with tile.TileContext(nc) as tc:
        with (
            tc.tile_pool(name="const", bufs=1) as cpool,
            tc.tile_pool(name="pkstg", bufs=3) as pkstg,
            tc.tile_pool(name="wa32", bufs=3) as wa32p,
            tc.tile_pool(name="w16", bufs=4) as w16p,
            tc.tile_pool(name="wc1p", bufs=8) as wc1p,
            tc.tile_pool(name="vstp", bufs=6) as vstp,
            tc.tile_pool(name="dec16p", bufs=2) as dec16p,
            tc.tile_pool(name="dec", bufs=3) as dec_pool,
            tc.tile_pool(name="sm", bufs=4) as sm_pool,
            tc.tile_pool(name="outp", bufs=2) as out_pool,
            tc.tile_pool(name="psA", bufs=2, space="PSUM") as psA,
            tc.tile_pool(name="psB", bufs=6, space="PSUM") as psB,
        ):
            # ---------------- resident tensors ----------------
            enc32 = cpool.tile([128, HC, B * W], F32R)
            Wc2T_sb = cpool.tile([128, HC, H], F32R)
            consts_sb = cpool.tile([3 * W, B + 3], F32)
            ones_sb = cpool.tile([W, W], F32R)
            uT_sb = cpool.tile([128, HC, B * W], F32R)
            # zero-padded align stacks (plane o = batches with n%3 == o):
            # ctx contracts over a whole v group; batch n's 33 softmax rows
            # sit at partition (n%3)*W of plane n%3, zero rows cancel the
            # other batches' windows exactly.
            t3_sb = cpool.tile([3 * W, 3, Q], F32R)
            bias_ap = consts_sb[0:W, 0:B]

            pk_r = pk16[:, :].rearrange("(c p) m -> p c m", p=128)
            Wc2_r = W_c2T[:, :].rearrange("(c p) m -> p c m", p=128)
            Wc1_r = W_c1T[:, :].rearrange("(cp p) m -> p cp m", p=128)
            dec_r = dec_hT[:, :].rearrange("(c p) (n q) -> p c n q", p=128, q=Q)
            zeros_r = zerosD[:, :].rearrange("p (o q) -> p o q", q=Q)

            # ---------------- DMA kickoff (issue order == priority) -------
            # chunk 0 lands in three pieces so the very first u matmul
            # (W_aT cols 0:128 + enc cols) can start ~2us earlier
            pk_stage = {}
            st0 = pkstg.tile([128, PK], F16, tag="pk", name="pk0")
            nc.sync.dma_start(out=st0[:, 0:128], in_=pk_r[:, 0, 0:128])
            nc.gpsimd.dma_start(out=st0[:, H:PK], in_=pk_r[:, 0, H:PK])
            nc.scalar.dma_start(out=st0[:, 128:H], in_=pk_r[:, 0, 128:H])
            pk_stage[0] = st0
            qname = {0: nc.sync, 1: nc.gpsimd, 2: nc.scalar}
            pk_q = [None, 0, 1, 2, 0, 1, 2, 1]  # queue per chunk
            for kc in range(1, HC):
                st = pkstg.tile([128, PK], F16, tag="pk", name=f"pk{kc}")
                qname[pk_q[kc]].dma_start(out=st, in_=pk_r[:, kc, :])
                pk_stage[kc] = st
            nc.gpsimd.dma_start(out=consts_sb, in_=constsD[:, :])
            nc.gpsimd.dma_start(out=ones_sb, in_=onesD[:, :])

            # dec batches 0-1 on the scalar queue (needed ~29us)
            dec16_tiles = {}
            for n in range(2):
                d16 = dec16p.tile([128, HC, Q], F16, tag="d16", name=f"d16_{n}")
                nc.scalar.dma_start(out=d16, in_=dec_r[:, :, n, :])
                dec16_tiles[n] = d16

            # W_c1 fp16 stages, kcp-granular, split sync/gpsimd (v ~20-27us)
            wc1_stage = {}
            for nt in range(2):
                for kcp in range(4):
                    st = w16p.tile([128, 2, 512], F16, tag="w16",
                                   name=f"wc1s_{nt}_{kcp}")
                    eng = nc.sync if kcp % 2 == 0 else nc.gpsimd
                    eng.dma_start(
                        out=st,
                        in_=Wc1_r[:, 2 * kcp:2 * kcp + 2, nt * 512:(nt + 1) * 512],
                    )
                    wc1_stage[(nt, kcp)] = st
            nc.sync.dma_start(out=t3_sb[:, :, :], in_=zeros_r)

            # W_c2 fp16 quarter stages (consumed ~29-35us)
            wc2_stage = {}
            wc2_q = [nc.scalar, nc.gpsimd, nc.sync, nc.gpsimd]
            for qtr in range(4):
                st = w16p.tile([128, HC, 256], F16, tag="w16", name=f"wc2s_{qtr}")
                wc2_q[qtr].dma_start(out=st, in_=Wc2_r[:, :, qtr * 256:(qtr + 1) * 256])
                wc2_stage[qtr] = st

            # ---------------- u phase: uT[h, (n,j)], kc-outer, f32r -------
            # upcast each chunk as it lands (W_aT part into a rotating
            # buffer, enc part into its keeper), stream through all 8
            # output chunks / PSUM banks.
            pu = {}
            for kc in range(HC):
                wa32 = wa32p.tile([128, H], F32R, tag="wa", name=f"wa32_{kc}")
                if kc == 0:
                    nc.vector.tensor_copy(out=wa32[:, 0:128], in_=pk_stage[0][:, 0:128])
                    nc.vector.tensor_copy(out=enc32[:, 0, :], in_=pk_stage[0][:, H:PK])
                    nc.vector.tensor_copy(out=wa32[:, 128:H], in_=pk_stage[0][:, 128:H])
                else:
                    nc.vector.tensor_copy(out=wa32, in_=pk_stage[kc][:, 0:H])
                    nc.vector.tensor_copy(out=enc32[:, kc, :], in_=pk_stage[kc][:, H:PK])
                for hc in range(HC):
                    if kc == 0:
                        pool = psB if hc < 6 else psA
                        tag = "B" if hc < 6 else "A"
                        pu[hc] = pool.tile([128, B * W], F32, tag=tag, name=f"pu{hc}")
                    nc.tensor.matmul(
                        pu[hc],
                        lhsT=wa32[:, hc * 128:(hc + 1) * 128],
                        rhs=enc32[:, kc, :],
                        start=(kc == 0),
                        stop=(kc == HC - 1),
                    )
            for hc in range(HC):
                if hc % 2 == 0:
                    nc.vector.tensor_copy(out=uT_sb[:, hc, :], in_=pu[hc])
                else:
                    nc.scalar.copy(out=uT_sb[:, hc, :], in_=pu[hc])

            # W_c1 upcasts in consumption order (scalar takes even kcp)
            wc1_tiles = {}
            for nt in range(2):
                for kcp in range(4):
                    wt = wc1p.tile([128, 2, 512], F32R, tag="wc1",
                                   name=f"wc1_{nt}_{kcp}")
                    if kcp % 2 == 0:
                        nc.scalar.copy(out=wt, in_=wc1_stage[(nt, kcp)])
                    else:
                        nc.vector.tensor_copy(out=wt, in_=wc1_stage[(nt, kcp)])
                    wc1_tiles[(nt, kcp)] = wt

            # ---------------- v phase: v[(n,j), h'] = enc_w @ W_c1T -------
            # vst tiles stay resident; ctx contracts over the whole group.
            vst_tiles = {}
            for nt in range(2):
                for gi in range(3):
                    g0, glen = GROUPS[gi]
                    pv = psB.tile([128, 512], F32, tag="B", name=f"pv{nt}_{gi}")
                    for kcp in range(4):
                        for j in range(2):
                            kc = 2 * kcp + j
                            nc.tensor.matmul(
                                pv[:glen, :],
                                lhsT=enc32[:, kc, g0:g0 + glen],
                                rhs=wc1_tiles[(nt, kcp)][:, j, :],
                                start=(kc == 0),
                                stop=(kc == HC - 1),
                            )
                    vst = vstp.tile([128, 512], F32R, tag="vst", name=f"vst{nt}_{gi}")
                    # evacuate + fold the gaussian in one op
                    gp = consts_sb[0:glen, B + gi:B + gi + 1]
                    if (nt + gi) % 2 == 0:
                        nc.vector.tensor_scalar_mul(vst[:glen, :], pv[:glen, :], gp)
                    else:
                        nc.scalar.activation(
                            out=vst[:glen, :], in_=pv[:glen, :], func=AF.Copy, scale=gp
                        )
                    vst_tiles[(nt, gi)] = vst

            # dec0 upcast, then W_c2 upcasts in consumption order
            dec32_tiles = {}

            def dec_cast(n):
                d16 = dec16_tiles.pop(n)
                if n + 2 < B and n + 2 not in dec16_tiles:
                    nxt = dec16p.tile([128, HC, Q], F16, tag="d16", name=f"d16_{n+2}")
                    nc.scalar.dma_start(out=nxt, in_=dec_r[:, :, n + 2, :])
                    dec16_tiles[n + 2] = nxt
                dec_sb = dec_pool.tile([128, HC, Q], F32R, tag="dec", name=f"dec{n}")
                nc.vector.tensor_copy(out=dec_sb, in_=d16)
                dec32_tiles[n] = dec_sb

            dec_cast(0)
            wc2_ceng = [nc.scalar, None, nc.vector, None]
            for qtr in range(4):
                dst = Wc2T_sb[:, :, qtr * 256:(qtr + 1) * 256]
                eng = wc2_ceng[qtr]
                if eng is None:
                    nc.gpsimd.tensor_copy(out=dst, in_=wc2_stage[qtr])
                elif eng is nc.scalar:
                    nc.scalar.copy(out=dst, in_=wc2_stage[qtr])
                else:
                    nc.vector.tensor_copy(out=dst, in_=wc2_stage[qtr])

            # ---------------- per-batch pipeline ----------------
            prev = None  # (n, pos, o_sb) awaiting tanh + store

            def flush_prev():
                nonlocal prev
                if prev is None:
                    return
                pn, ppos, po_sb = prev
                for qt in range(2):
                    for ht in range(2):
                        nc.scalar.activation(
                            out=po_sb[:, qt, ht * 512:(ht + 1) * 512],
                            in_=ppos[(qt, ht)], func=AF.Tanh,
                        )
                dst = out[pn * Q:(pn + 1) * Q, :].rearrange("(qt p) h -> p qt h", p=128)
                eng = nc.sync if pn % 2 == 0 else nc.gpsimd
                eng.dma_start(out=dst, in_=po_sb)
                prev = None

            state = {}
            scored = {}

            def score_part(n):
                if n not in dec32_tiles:
                    dec_cast(n)
                dec_sb = dec32_tiles.pop(n)
                ps = psA.tile([W, Q], F32, tag="A", name=f"ps{n}")
                for hc in range(HC):
                    nc.tensor.matmul(
                        ps,
                        lhsT=uT_sb[:, hc, n * W:(n + 1) * W],
                        rhs=dec_sb[:, hc, :],
                        start=(hc == 0),
                        stop=(hc == HC - 1),
                    )
                # softmax over 33 partitions via 4th-power renormalization:
                # t=exp(s/4+b); T=colsum t; t=(t/T)^4; Z=colsum t; t/=Z.
                t = sm_pool.tile([W, Q], F32R, tag="t", name=f"t{n}")
                nc.scalar.activation(
                    out=t, in_=ps, func=AF.Exp, bias=bias_ap[:, n:n + 1], scale=0.25
                )
                scored[n] = (dec_sb, t)

            def softmax_T(n, t):
                pT = psA.tile([W, Q], F32, tag="A", name=f"pT{n}")
                nc.tensor.matmul(pT, lhsT=ones_sb[:], rhs=t[:], start=True, stop=True)
                rT = sm_pool.tile([W, Q], F32, tag="rT", name=f"rT{n}")
                nc.vector.reciprocal_approx_fast(out=rT, in_=pT)
                nc.vector.tensor_mul(t, t, rT)
                nc.vector.tensor_mul(t, t, t)
                nc.vector.tensor_mul(t, t, t)

            def softmax_Z(n, t):
                pZ = psA.tile([W, Q], F32, tag="A", name=f"pZ{n}")
                nc.tensor.matmul(pZ, lhsT=ones_sb[:], rhs=t[:], start=True, stop=True)
                rZ = sm_pool.tile([W, Q], F32, tag="rZ", name=f"rZ{n}")
                nc.vector.reciprocal_approx_fast(out=rZ, in_=pZ)
                tf = sm_pool.tile([W, Q], F32R, tag="tf", name=f"tf{n}")
                nc.vector.tensor_mul(tf, t, rZ)
                off = n % 3
                eng = nc.sync if n % 2 == 0 else nc.gpsimd
                eng.dma_start(out=t3_sb[off * W:(off + 1) * W, off, :], in_=tf[:, :])

            def dec_group(n, dec_sb, pos, qt, ht, start=True, stop=False):
                if start:
                    po = psB.tile([128, 512], F32, tag="B", name=f"po{n}_{qt}_{ht}")
                    pos[(qt, ht)] = po
                else:
                    po = pos[(qt, ht)]
                for hc in range(HC):
                    nc.tensor.matmul(
                        po,
                        lhsT=dec_sb[:, hc, qt * 128:(qt + 1) * 128],
                        rhs=Wc2T_sb[:, hc, ht * 512:(ht + 1) * 512],
                        start=start and (hc == 0),
                        stop=stop and (hc == HC - 1),
                    )

            def ctx_mm(n, pos, qt, ht, stop=True):
                gi, off = divmod(n, 3)
                glen = GROUPS[gi][1]
                nc.tensor.matmul(
                    pos[(qt, ht)],
                    lhsT=t3_sb[0:glen, off, qt * 128:(qt + 1) * 128],
                    rhs=vst_tiles[(ht, gi)][0:glen, :],
                    start=False,
                    stop=stop,
                )

            def batch_pre(n):
                if n not in scored:
                    score_part(n)
                dec_sb, t = scored.pop(n)
                flush_prev()
                o_sb = out_pool.tile([128, 2, H], F16, tag="o", name=f"o{n}")
                pos = {}
                # ht=0 groups first: W_c2's upper half may still be landing
                dec_group(n, dec_sb, pos, 0, 0)
                softmax_T(n, t)
                dec_group(n, dec_sb, pos, 1, 0)
                dec_group(n, dec_sb, pos, 0, 1)
                softmax_Z(n, t)
                dec_group(n, dec_sb, pos, 1, 1)
                state[n] = (pos, o_sb)

            def batch_ctx(n):
                pos, o_sb = state.pop(n)
                for qt in range(2):
                    for ht in range(2):
                        ctx_mm(n, pos, qt, ht)
                nonlocal prev
                prev = (n, pos, o_sb)

            def batch_last(n):
                # softmax for n already ran during batch n-1; each W_c2
                # quarter closes with its ctx and drains tanh+store while
                # the next quarter streams.
                dec_sb, _ = scored.pop(n)
                flush_prev()
                o_sb = out_pool.tile([128, 2, H], F16, tag="o", name=f"o{n}")
                dst = out[n * Q:(n + 1) * Q, :].rearrange("(qt p) h -> p qt h", p=128)
                pos = {}

                def drain(qt, ht):
                    ctx_mm(n, pos, qt, ht)
                    nc.scalar.activation(
                        out=o_sb[:, qt, ht * 512:(ht + 1) * 512],
                        in_=pos[(qt, ht)], func=AF.Tanh,
                    )
                    eng = nc.sync if (qt + ht) % 2 == 0 else nc.gpsimd
                    eng.dma_start(
                        out=dst[:, qt, ht * 512:(ht + 1) * 512],
                        in_=o_sb[:, qt, ht * 512:(ht + 1) * 512],
                    )

                dec_group(n, dec_sb, pos, 0, 0)
                dec_group(n, dec_sb, pos, 0, 1)
                drain(0, 0)
                drain(0, 1)
                dec_group(n, dec_sb, pos, 1, 0)
                drain(1, 0)
                dec_group(n, dec_sb, pos, 1, 1)
                drain(1, 1)

            for n in range(B - 1):
                batch_pre(n)
                if n == B - 2:
                    # emit the last batch's score+softmax here so its t3 is
                    # ready before batch_last's interleaved ctx/drain
                    score_part(B - 1)
                    _, t_last = scored[B - 1]
                    softmax_T(B - 1, t_last)
                    softmax_Z(B - 1, t_last)
                batch_ctx(n)
            batch_last(B - 1)
    nc.compile()
    return nc


def prepare_in_maps(inputs: dict) -> list[dict]:
    enc = np.asarray(inputs["encoder_outputs"], dtype=np.float32)
    dec = np.asarray(inputs["decoder_h_t"], dtype=np.float32)
    src_len = np.asarray(inputs["src_len"], dtype=np.int32)
    p_t = np.asarray(inputs["p_t"], dtype=np.float32)
    W_a = np.asarray(inputs["W_a"], dtype=np.float32)
    W_c = np.asarray(inputs["W_c"], dtype=np.float32)

    # Window bounds, computed with the same fp32 ops as the reference.
    attn_start = np.maximum(p_t - np.float32(WINDOW), np.float32(0.0))
    attn_end = np.minimum(p_t + np.float32(WINDOW), src_len.astype(np.float32))
    s = np.ceil(attn_start).astype(np.int64)
    s = np.minimum(s, L - W)  # keep the 33-slice in bounds
    idx = s[:, None] + np.arange(W)[None, :]
    idxf = idx.astype(np.float32)
    mask = (idxf < attn_start[:, None]) | (idxf > attn_end[:, None])
    bias = np.where(mask, np.float32(MASK_BIAS), np.float32(LOG_ALPHA)).astype(np.float32)
    g = np.exp(-((idxf - p_t[:, None]) ** 2) / np.float32(DEV_POW)).astype(np.float32)

    enc_w = enc[np.arange(N)[:, None], idx, :].astype(np.float16)  # [N, W, H]
    dec = dec.astype(np.float16)
    W_aT16 = W_a.T.astype(np.float16)
    W_c1T = W_c[:, :H].T.astype(np.float16)
    W_c2T = W_c[:, H:].T.astype(np.float16)

    in_maps = []
    for c in range(NCORES):
        bs = slice(c * B, (c + 1) * B)
        enc_wT = enc_w[bs].transpose(2, 0, 1).reshape(H, B * W)  # [H, B*W]
        pk = np.concatenate([W_aT16, enc_wT], axis=1)            # [H, H+B*W]
        gc = g[bs]  # [B, W]
        consts = np.zeros((3 * W, B + 3), dtype=np.float32)
        consts[0:W, 0:B] = bias[bs].T
        for n in range(B):
            gi, off = divmod(n, 3)
            consts[off * W:(off + 1) * W, B + gi] = gc[n]
        in_maps.append({
            "pk16": np.ascontiguousarray(pk),
            "dec_hT": np.ascontiguousarray(dec[bs].transpose(2, 0, 1).reshape(H, B * Q)),
            "W_c1T": W_c1T,
            "W_c2T": W_c2T,
            "constsD": consts,
            "onesD": np.ones((W, W), dtype=np.float32),
            "zerosD": np.zeros((3 * W, 3 * Q), dtype=np.float32),
        })
    return in_maps


_NC = None


def get_nc() -> bass.Bass:
    global _NC
    if _NC is None:
        _NC = build_nc()
    return _NC


def kernel(**inputs) -> np.ndarray:
    nc = get_nc()
    in_maps = prepare_in_maps(inputs)
    res = run_bass_kernel_spmd(nc, in_maps, list(range(NCORES)))
    outs = [np.asarray(res.results[c]["out"], dtype=np.float32).reshape(B, Q, H)
            for c in range(NCORES)]
    return np.concatenate(outs, axis=0)


# revision 28
# speedup vs baseline: 1.1971x; 1.0078x over previous
"""Trainium2 Bass kernel for Luong local-p sparse attention.

Math (per batch n, full shapes N=64, L=258, H=1024, Q=256):
    score = (h_t @ W_a) @ enc^T           masked to window [p_t-16, p_t+16]
    align = softmax(score) * gauss(p_t)
    out   = tanh([align @ enc, h_t] @ W_c^T)

Only a 33-wide window of enc survives the mask, so windows are gathered
host-side and W_a / W_c[:, :H] are pushed through the 33-wide side:
    u  = W_a-transform of window   (uT[h, (n,j)]  = sum_k W_aT[k,h] enc_w[(n,j),k])
    s  = uT^T-partial scores       (score^T[j, q] = sum_h uT[h,j] h_t[q,h])
    softmax over j (33 rows) done j-major with a 4th-power renormalization
    trick (no partition-dim max needed; partition sums via ones-matmul)
    v  = W_c1-transform of window  (v[(n,j), h']  = sum_h enc_w[(n,j),h] W_c1T[h,h'])
    out = tanh(h_t @ W_c2T + align-stack @ v)

All matmuls run float32r (fp32_mode=HIGH streams 1 row / 2.4GHz cycle;
fp16/bf16 only stream at 2.0GHz, measured).  Everything heavy travels
the wire as fp16 (the same 11-bit mantissa the PE keeps in f32r mode)
and is upcast on-chip, halving load DMA.  Per-engine instruction queues
are strictly in-order, so every cast/exp/DMA-issue is emitted in the
order its inputs arrive — an early-emitted late-input op would block
the whole queue (measured 3-5us priority-inversion stalls otherwise).

The u phase runs kc-outer across all 8 PSUM banks so the PE starts on
the first W_aT chunk; the v phase follows while W_c2 still streams in.
ctx contracts over a whole 99-row v group with a zero-padded align
stack (t3): batch n's 33 softmax rows sit at partition (n%3)*W, zero
rows cancel the other windows, so the v output never needs
partition-shuffle DMAs and stays matmul-aligned (matmul operands must
start at partition 0/32/64).  The last batch's softmax runs during
batch 6 and its ctx/tanh/store drain per W_c2 quarter to cut the tail.

Data parallel over batch: 8 batches per core x 8 cores.  Output is
stored fp16 (tanh output in [-1,1]; host upcasts).
"""

import numpy as np

import concourse.bass as bass
import concourse.bacc as bacc
import concourse.mybir as mybir
import concourse.tile as tile
from concourse.bass_utils import run_bass_kernel_spmd

# Problem constants (hardcoded per harness contract).
N, L, H, Q = 64, 258, 1024, 256
WINDOW = 16.0
DEV_POW = 128.0
NCORES = 8
B = N // NCORES  # batches per core
W = 33           # window width (positions that can survive the mask)
HC = H // 128    # h-chunks of 128 (PE contraction tiles)
PK = H + B * W   # packed [W_aT | enc_wT] row width
F32 = mybir.dt.float32
F32R = mybir.dt.float32r
F16 = mybir.dt.float16
AF = mybir.ActivationFunctionType

# exp is computed as t = exp(s/4 + bias); bias = LOG_ALPHA keeps the
# column-sum T = sum_j t below fp32 max.  alpha cancels in w = t/T.
LOG_ALPHA = -4.8520302  # -7*ln(2)
MASK_BIAS = -10000.0    # exp(<= -9900) == 0 in fp32

# v-phase window groups: batches gi*3+off live at partitions off*W..off*W+32
GROUPS = [(0, 99), (99, 99), (198, 66)]


def build_nc() -> bass.Bass:
    nc = bacc.Bacc()
    pk16 = nc.declare_dram_parameter("pk16", [H, PK], F16, isOutput=False)
    dec_hT = nc.declare_dram_parameter("dec_hT", [H, B * Q], F16, isOutput=False)
    W_c1T = nc.declare_dram_parameter("W_c1T", [H, H], F16, isOutput=False)
    W_c2T = nc.declare_dram_parameter("W_c2T", [H, H], F16, isOutput=False)
    constsD = nc.declare_dram_parameter("constsD", [3 * W, B + 3], F32, isOutput=False)
    onesD = nc.declare_dram_parameter("onesD", [W, W], F32R, isOutput=False)
    zerosD = nc.declare_dram_parameter("zerosD", [3 * W, 3 * Q], F32R, isOutput=False)
    out = nc.declare_dram_parameter("out", [B * Q, H], F16, isOutput=True)

    with tile.TileContext(nc) as tc:
        with (
            tc.tile_pool(name="const", bufs=1) as cpool,
            tc.tile_pool(name="pkstg", bufs=3) as pkstg,
            tc.tile_pool(name="wa32", bufs=3) as wa32p,
            tc.tile_pool(name="w16", bufs=4) as w16p,
            tc.tile_pool(name="wc1p", bufs=8) as wc1p,
            tc.tile_pool(name="vstp", bufs=6) as vstp,
            tc.tile_pool(name="dec16p", bufs=2) as dec16p,
            tc.tile_pool(name="dec", bufs=3) as dec_pool,
            tc.tile_pool(name="sm", bufs=4) as sm_pool,
            tc.tile_pool(name="outp", bufs=2) as out_pool,
            tc.tile_pool(name="psA", bufs=2, space="PSUM") as psA,
            tc.tile_pool(name="psB", bufs=6, space="PSUM") as psB,
        ):
            # ---------------- resident tensors ----------------
            enc32 = cpool.tile([128, HC, B * W], F32R)
            Wc2T_sb = cpool.tile([128, HC, H], F32R)
            consts_sb = cpool.tile([3 * W, B + 3], F32)
            ones_sb = cpool.tile([W, W], F32R)
            uT_sb = cpool.tile([128, HC, B * W], F32R)
            # zero-padded align stacks (plane o = batches with n%3 == o):
            # ctx contracts over a whole v group; batch n's 33 softmax rows
            # sit at partition (n%3)*W of plane n%3, zero rows cancel the
            # other batches' windows exactly.
            t3_sb = cpool.tile([3 * W, 3, Q], F32R)
            bias_ap = consts_sb[0:W, 0:B]

            pk_r = pk16[:, :].rearrange("(c p) m -> p c m", p=128)
            Wc2_r = W_c2T[:, :].rearrange("(c p) m -> p c m", p=128)
            Wc1_r = W_c1T[:, :].rearrange("(cp p) m -> p cp m", p=128)
            dec_r = dec_hT[:, :].rearrange("(c p) (n q) -> p c n q", p=128, q=Q)
            zeros_r = zerosD[:, :].rearrange("p (o q) -> p o q", q=Q)

            # ---------------- DMA kickoff (issue order == priority) -------
            # chunk 0 lands in three pieces so the very first u matmul
            # (W_aT cols 0:128 + enc cols) can start ~2us earlier
            # each queue's items are ordered by consumption deadline;
            # per-queue bandwidth is ~105GB/s so balance bytes too
            pk_stage = {}
            st0 = pkstg.tile([128, PK], F16, tag="pk", name="pk0")
            nc.sync.dma_start(out=st0[:, 0:128], in_=pk_r[:, 0, 0:128])
            nc.gpsimd.dma_start(out=st0[:, H:PK], in_=pk_r[:, 0, H:PK])
            nc.scalar.dma_start(out=st0[:, 128:H], in_=pk_r[:, 0, 128:H])
            pk_stage[0] = st0
            qname = {0: nc.sync, 1: nc.gpsimd, 2: nc.scalar}
            pk_q = [None, 0, 1, 2, 0, 1, 2, 0]  # queue per chunk
            for kc in range(1, HC):
                st = pkstg.tile([128, PK], F16, tag="pk", name=f"pk{kc}")
                qname[pk_q[kc]].dma_start(out=st, in_=pk_r[:, kc, :])
                pk_stage[kc] = st
            nc.gpsimd.dma_start(out=consts_sb, in_=constsD[:, :])
            nc.gpsimd.dma_start(out=ones_sb, in_=onesD[:, :])

            # W_c1 fp16 stages, kcp-granular, in v-phase consumption order
            wc1_stage = {}
            wc1_q = {(0, 0): nc.sync, (0, 1): nc.gpsimd, (0, 2): nc.scalar,
                     (0, 3): nc.gpsimd, (1, 0): nc.sync, (1, 1): nc.gpsimd,
                     (1, 2): nc.scalar, (1, 3): nc.gpsimd}
            for nt in range(2):
                for kcp in range(4):
                    st = w16p.tile([128, 2, 512], F16, tag="w16",
                                   name=f"wc1s_{nt}_{kcp}")
                    wc1_q[(nt, kcp)].dma_start(
                        out=st,
                        in_=Wc1_r[:, 2 * kcp:2 * kcp + 2, nt * 512:(nt + 1) * 512],
                    )
                    wc1_stage[(nt, kcp)] = st

            # dec batch 0 (needed ~27us), then W_c2 quarters (~31-37us)
            dec16_tiles = {}
            d16 = dec16p.tile([128, HC, Q], F16, tag="d16", name="d16_0")
            nc.scalar.dma_start(out=d16, in_=dec_r[:, :, 0, :])
            dec16_tiles[0] = d16
            wc2_stage = {}
            wc2_q = [nc.sync, nc.gpsimd, nc.scalar, nc.gpsimd]
            for qtr in range(4):
                st = w16p.tile([128, HC, 256], F16, tag="w16", name=f"wc2s_{qtr}")
                wc2_q[qtr].dma_start(out=st, in_=Wc2_r[:, :, qtr * 256:(qtr + 1) * 256])
                wc2_stage[qtr] = st
            nc.sync.dma_start(out=t3_sb[:, :, :], in_=zeros_r)
            d16 = dec16p.tile([128, HC, Q], F16, tag="d16", name="d16_1")
            nc.scalar.dma_start(out=d16, in_=dec_r[:, :, 1, :])
            dec16_tiles[1] = d16

            # ---------------- u phase: uT[h, (n,j)], kc-outer, f32r -------
            # upcast each chunk as it lands (W_aT part into a rotating
            # buffer, enc part into its keeper), stream through all 8
            # output chunks / PSUM banks.
            pu = {}
            for kc in range(HC):
                wa32 = wa32p.tile([128, H], F32R, tag="wa", name=f"wa32_{kc}")
                if kc == 0:
                    nc.vector.tensor_copy(out=wa32[:, 0:128], in_=pk_stage[0][:, 0:128])
                    nc.vector.tensor_copy(out=enc32[:, 0, :], in_=pk_stage[0][:, H:PK])
                    nc.vector.tensor_copy(out=wa32[:, 128:H], in_=pk_stage[0][:, 128:H])
                else:
                    nc.vector.tensor_copy(out=wa32, in_=pk_stage[kc][:, 0:H])
                    nc.vector.tensor_copy(out=enc32[:, kc, :], in_=pk_stage[kc][:, H:PK])
                for hc in range(HC):
                    if kc == 0:
                        pool = psB if hc < 6 else psA
                        tag = "B" if hc < 6 else "A"
                        pu[hc] = pool.tile([128, B * W], F32, tag=tag, name=f"pu{hc}")
                    nc.tensor.matmul(
                        pu[hc],
                        lhsT=wa32[:, hc * 128:(hc + 1) * 128],
                        rhs=enc32[:, kc, :],
                        start=(kc == 0),
                        stop=(kc == HC - 1),
                    )
            # Evacuate pu0/pu1 first (the v phase recycles their PSUM banks
            # immediately), then W_c1 upcasts in consumption order (scalar
            # takes even kcp), then the remaining evacuations — the later
            # evacs aren't needed until the first score (~31us) while v
            # wants wc1 at ~21us, and each engine queue is in-order.
            def evac(hc):
                if hc % 2 == 0:
                    nc.vector.tensor_copy(out=uT_sb[:, hc, :], in_=pu[hc])
                else:
                    nc.scalar.copy(out=uT_sb[:, hc, :], in_=pu[hc])

            evac(0)
            evac(1)
            wc1_tiles = {}
            for nt in range(2):
                for kcp in range(4):
                    wt = wc1p.tile([128, 2, 512], F32R, tag="wc1",
                                   name=f"wc1_{nt}_{kcp}")
                    if kcp % 2 == 0:
                        nc.scalar.copy(out=wt, in_=wc1_stage[(nt, kcp)])
                    else:
                        nc.vector.tensor_copy(out=wt, in_=wc1_stage[(nt, kcp)])
                    wc1_tiles[(nt, kcp)] = wt
            for hc in range(2, HC):
                evac(hc)

            # ---------------- v phase: v[(n,j), h'] = enc_w @ W_c1T -------
            # vst tiles stay resident; ctx contracts over the whole group.
            vst_tiles = {}
            for nt in range(2):
                for gi in range(3):
                    g0, glen = GROUPS[gi]
                    pv = psB.tile([128, 512], F32, tag="B", name=f"pv{nt}_{gi}")
                    for kcp in range(4):
                        for j in range(2):
                            kc = 2 * kcp + j
                            nc.tensor.matmul(
                                pv[:glen, :],
                                lhsT=enc32[:, kc, g0:g0 + glen],
                                rhs=wc1_tiles[(nt, kcp)][:, j, :],
                                start=(kc == 0),
                                stop=(kc == HC - 1),
                            )
                    vst = vstp.tile([128, 512], F32R, tag="vst", name=f"vst{nt}_{gi}")
                    # evacuate + fold the gaussian in one op
                    gp = consts_sb[0:glen, B + gi:B + gi + 1]
                    if (nt + gi) % 2 == 0:
                        nc.vector.tensor_scalar_mul(vst[:glen, :], pv[:glen, :], gp)
                    else:
                        nc.scalar.activation(
                            out=vst[:glen, :], in_=pv[:glen, :], func=AF.Copy, scale=gp
                        )
                    vst_tiles[(nt, gi)] = vst

            # dec0 upcast (on gpsimd: vector is still busy with gfolds),
            # then W_c2 upcasts in consumption order
            dec32_tiles = {}

            def dec_cast(n, eng=None):
                d16 = dec16_tiles.pop(n)
                if n + 2 < B and n + 2 not in dec16_tiles:
                    nxt = dec16p.tile([128, HC, Q], F16, tag="d16", name=f"d16_{n+2}")
                    nc.scalar.dma_start(out=nxt, in_=dec_r[:, :, n + 2, :])
                    dec16_tiles[n + 2] = nxt
                dec_sb = dec_pool.tile([128, HC, Q], F32R, tag="dec", name=f"dec{n}")
                (eng or nc.vector).tensor_copy(out=dec_sb, in_=d16)
                dec32_tiles[n] = dec_sb

            dec_cast(0, eng=nc.gpsimd)
            wc2_ceng = [nc.scalar, None, nc.vector, None]
            for qtr in range(4):
                dst = Wc2T_sb[:, :, qtr * 256:(qtr + 1) * 256]
                eng = wc2_ceng[qtr]
                if eng is None:
                    nc.gpsimd.tensor_copy(out=dst, in_=wc2_stage[qtr])
                elif eng is nc.scalar:
                    nc.scalar.copy(out=dst, in_=wc2_stage[qtr])
                else:
                    nc.vector.tensor_copy(out=dst, in_=wc2_stage[qtr])

            # ---------------- per-batch pipeline ----------------
            prev = None  # (n, pos, o_sb) awaiting tanh + store

            def flush_prev():
                nonlocal prev
                if prev is None:
                    return
                pn, ppos, po_sb = prev
                for qt in range(2):
                    for ht in range(2):
                        nc.scalar.activation(
                            out=po_sb[:, qt, ht * 512:(ht + 1) * 512],
                            in_=ppos[(qt, ht)], func=AF.Tanh,
                        )
                dst = out[pn * Q:(pn + 1) * Q, :].rearrange("(qt p) h -> p qt h", p=128)
                eng = nc.sync if pn % 2 == 0 else nc.gpsimd
                eng.dma_start(out=dst, in_=po_sb)
                prev = None

            state = {}
            scored = {}

            def score_part(n):
                if n not in dec32_tiles:
                    dec_cast(n)
                dec_sb = dec32_tiles.pop(n)
                ps = psA.tile([W, Q], F32, tag="A", name=f"ps{n}")
                for hc in range(HC):
                    nc.tensor.matmul(
                        ps,
                        lhsT=uT_sb[:, hc, n * W:(n + 1) * W],
                        rhs=dec_sb[:, hc, :],
                        start=(hc == 0),
                        stop=(hc == HC - 1),
                    )
                # softmax over 33 partitions via 4th-power renormalization:
                # t=exp(s/4+b); T=colsum t; t=(t/T)^4; Z=colsum t; t/=Z.
                t = sm_pool.tile([W, Q], F32R, tag="t", name=f"t{n}")
                nc.scalar.activation(
                    out=t, in_=ps, func=AF.Exp, bias=bias_ap[:, n:n + 1], scale=0.25
                )
                scored[n] = (dec_sb, t)

            def softmax_T(n, t):
                pT = psA.tile([W, Q], F32, tag="A", name=f"pT{n}")
                nc.tensor.matmul(pT, lhsT=ones_sb[:], rhs=t[:], start=True, stop=True)
                rT = sm_pool.tile([W, Q], F32, tag="rT", name=f"rT{n}")
                nc.vector.reciprocal_approx_fast(out=rT, in_=pT)
                nc.vector.tensor_mul(t, t, rT)
                nc.vector.tensor_mul(t, t, t)
                nc.vector.tensor_mul(t, t, t)

            def softmax_Z(n, t):
                pZ = psA.tile([W, Q], F32, tag="A", name=f"pZ{n}")
                nc.tensor.matmul(pZ, lhsT=ones_sb[:], rhs=t[:], start=True, stop=True)
                rZ = sm_pool.tile([W, Q], F32, tag="rZ", name=f"rZ{n}")
                nc.vector.reciprocal_approx_fast(out=rZ, in_=pZ)
                tf = sm_pool.tile([W, Q], F32R, tag="tf", name=f"tf{n}")
                nc.vector.tensor_mul(tf, t, rZ)
                off = n % 3
                eng = nc.sync if n % 2 == 0 else nc.gpsimd
                eng.dma_start(out=t3_sb[off * W:(off + 1) * W, off, :], in_=tf[:, :])

            def dec_group(n, dec_sb, pos, qt, ht, start=True, stop=False):
                if start:
                    po = psB.tile([128, 512], F32, tag="B", name=f"po{n}_{qt}_{ht}")
                    pos[(qt, ht)] = po
                else:
                    po = pos[(qt, ht)]
                for hc in range(HC):
                    nc.tensor.matmul(
                        po,
                        lhsT=dec_sb[:, hc, qt * 128:(qt + 1) * 128],
                        rhs=Wc2T_sb[:, hc, ht * 512:(ht + 1) * 512],
                        start=start and (hc == 0),
                        stop=stop and (hc == HC - 1),
                    )

            def ctx_mm(n, pos, qt, ht, stop=True):
                gi, off = divmod(n, 3)
                glen = GROUPS[gi][1]
                nc.tensor.matmul(
                    pos[(qt, ht)],
                    lhsT=t3_sb[0:glen, off, qt * 128:(qt + 1) * 128],
                    rhs=vst_tiles[(ht, gi)][0:glen, :],
                    start=False,
                    stop=stop,
                )

            def batch_pre(n):
                if n not in scored:
                    score_part(n)
                dec_sb, t = scored.pop(n)
                flush_prev()
                o_sb = out_pool.tile([128, 2, H], F16, tag="o", name=f"o{n}")
                pos = {}
                # ht=0 groups first: W_c2's upper half may still be landing
                dec_group(n, dec_sb, pos, 0, 0)
                softmax_T(n, t)
                dec_group(n, dec_sb, pos, 1, 0)
                dec_group(n, dec_sb, pos, 0, 1)
                softmax_Z(n, t)
                dec_group(n, dec_sb, pos, 1, 1)
                state[n] = (pos, o_sb)

            def batch_ctx(n):
                pos, o_sb = state.pop(n)
                for qt in range(2):
                    for ht in range(2):
                        ctx_mm(n, pos, qt, ht)
                nonlocal prev
                prev = (n, pos, o_sb)

            def batch_last(n):
                # softmax for n already ran during batch n-1; each W_c2
                # quarter closes with its ctx and drains tanh+store while
                # the next quarter streams.
                dec_sb, _ = scored.pop(n)
                flush_prev()
                o_sb = out_pool.tile([128, 2, H], F16, tag="o", name=f"o{n}")
                dst = out[n * Q:(n + 1) * Q, :].rearrange("(qt p) h -> p qt h", p=128)
                pos = {}

                def drain(qt, ht):
                    ctx_mm(n, pos, qt, ht)
                    nc.scalar.activation(
                        out=o_sb[:, qt, ht * 512:(ht + 1) * 512],
                        in_=pos[(qt, ht)], func=AF.Tanh,
                    )
                    eng = nc.sync if (qt + ht) % 2 == 0 else nc.gpsimd
                    eng.dma_start(
                        out=dst[:, qt, ht * 512:(ht + 1) * 512],
                        in_=o_sb[:, qt, ht * 512:(ht + 1) * 512],
                    )

                dec_group(n, dec_sb, pos, 0, 0)
                dec_group(n, dec_sb, pos, 0, 1)
                drain(0, 0)
                drain(0, 1)
                dec_group(n, dec_sb, pos, 1, 0)
                drain(1, 0)
                dec_group(n, dec_sb, pos, 1, 1)
                drain(1, 1)

            for n in range(B - 1):
                batch_pre(n)
                if n == B - 2:
                    # emit the last batch's score+softmax here so its t3 is
                    # ready before batch_last's interleaved ctx/drain
                    score_part(B - 1)
                    _, t_last = scored[B - 1]
                    softmax_T(B - 1, t_last)
                    softmax_Z(B - 1, t_last)
                batch_ctx(n)
            batch_last(B - 1)
    nc.compile()
    return nc


def prepare_in_maps(inputs: dict) -> list[dict]:
    enc = np.asarray(inputs["encoder_outputs"], dtype=np.float32)
    dec = np.asarray(inputs["decoder_h_t"], dtype=np.float32)
    src_len = np.asarray(inputs["src_len"], dtype=np.int32)
    p_t = np.asarray(inputs["p_t"], dtype=np.float32)
    W_a = np.asarray(inputs["W_a"], dtype=np.float32)
    W_c = np.asarray(inputs["W_c"], dtype=np.float32)

    # Window bounds, computed with the same fp32 ops as the reference.
    attn_start = np.maximum(p_t - np.float32(WINDOW), np.float32(0.0))
    attn_end = np.minimum(p_t + np.float32(WINDOW), src_len.astype(np.float32))
    s = np.ceil(attn_start).astype(np.int64)
    s = np.minimum(s, L - W)  # keep the 33-slice in bounds
    idx = s[:, None] + np.arange(W)[None, :]
    idxf = idx.astype(np.float32)
    mask = (idxf < attn_start[:, None]) | (idxf > attn_end[:, None])
    bias = np.where(mask, np.float32(MASK_BIAS), np.float32(LOG_ALPHA)).astype(np.float32)
    g = np.exp(-((idxf - p_t[:, None]) ** 2) / np.float32(DEV_POW)).astype(np.float32)

    enc_w = enc[np.arange(N)[:, None], idx, :].astype(np.float16)  # [N, W, H]
    dec = dec.astype(np.float16)
    W_aT16 = W_a.T.astype(np.float16)
    W_c1T = W_c[:, :H].T.astype(np.float16)
    W_c2T = W_c[:, H:].T.astype(np.float16)

    in_maps = []
    for c in range(NCORES):
        bs = slice(c * B, (c + 1) * B)
        enc_wT = enc_w[bs].transpose(2, 0, 1).reshape(H, B * W)  # [H, B*W]
        pk = np.concatenate([W_aT16, enc_wT], axis=1)            # [H, H+B*W]
        gc = g[bs]  # [B, W]
        consts = np.zeros((3 * W, B + 3), dtype=np.float32)
        consts[0:W, 0:B] = bias[bs].T
        for n in range(B):
            gi, off = divmod(n, 3)
            consts[off * W:(off + 1) * W, B + gi] = gc[n]
        in_maps.append({
            "pk16": np.ascontiguousarray(pk),
            "dec_hT": np.ascontiguousarray(dec[bs].transpose(2, 0, 1).reshape(H, B * Q)),
            "W_c1T": W_c1T,
            "W_c2T": W_c2T,
            "constsD": consts,
            "onesD": np.ones((W, W), dtype=np.float32),
            "zerosD": np.zeros((3 * W, 3 * Q), dtype=np.float32),
        })
    return in_maps


_NC = None


def get_nc() -> bass.Bass:
    global _NC
    if _NC is None:
        _NC = build_nc()
    return _NC


def kernel(**inputs) -> np.ndarray:
    nc = get_nc()
    in_maps = prepare_in_maps(inputs)
    res = run_bass_kernel_spmd(nc, in_maps, list(range(NCORES)))
    outs = [np.asarray(res.results[c]["out"], dtype=np.float32).reshape(B, Q, H)
            for c in range(NCORES)]
    return np.concatenate(outs, axis=0)


# revision 33
# speedup vs baseline: 1.2473x; 1.0420x over previous
"""Trainium2 Bass kernel for Luong local-p sparse attention.

Math (per batch n, full shapes N=64, L=258, H=1024, Q=256):
    score = (h_t @ W_a) @ enc^T           masked to window [p_t-16, p_t+16]
    align = softmax(score) * gauss(p_t)
    out   = tanh([align @ enc, h_t] @ W_c^T)

Only a 33-wide window of enc survives the mask, so windows are gathered
host-side and W_a / W_c[:, :H] are pushed through the 33-wide side:
    u  = W_a-transform of window   (uT[h, (n,j)]  = sum_k W_aT[k,h] enc_w[(n,j),k])
    s  = uT^T-partial scores       (score^T[j, q] = sum_h uT[h,j] h_t[q,h])
    softmax over j (33 rows) done j-major with a 4th-power renormalization
    trick (no partition-dim max needed; partition sums via ones-matmul)
    v  = W_c1-transform of window  (v[(n,j), h']  = sum_h enc_w[(n,j),h] W_c1T[h,h'])
    out = tanh(h_t @ W_c2T + align-stack @ v)

All matmuls run float32r (fp32_mode=HIGH streams 1 row / 2.4GHz cycle;
fp16/bf16 only stream at 2.0GHz, measured).  Everything heavy travels
the wire as fp16 (the same 11-bit mantissa the PE keeps in f32r mode)
and is upcast on-chip, halving load DMA.  Per-engine instruction queues
are strictly in-order, so every cast/exp/DMA-issue is emitted in the
order its inputs arrive — an early-emitted late-input op would block
the whole queue (measured 3-5us priority-inversion stalls otherwise).

The u phase runs kc-outer across all 8 PSUM banks so the PE starts on
the first W_aT chunk; the v phase follows while W_c2 still streams in.
ctx contracts over a whole 99-row v group with a zero-padded align
stack (t3): batch n's 33 softmax rows sit at partition (n%3)*W, zero
rows cancel the other windows, so the v output never needs
partition-shuffle DMAs and stays matmul-aligned (matmul operands must
start at partition 0/32/64).  The last batch's softmax runs during
batch 6 and its ctx/tanh/store drain per W_c2 quarter to cut the tail.

Data parallel over batch: 8 batches per core x 8 cores.  Output is
stored fp16 (tanh output in [-1,1]; host upcasts).
"""

import numpy as np

import concourse.bass as bass
import concourse.bacc as bacc
import concourse.mybir as mybir
import concourse.tile as tile
from concourse.bass_utils import run_bass_kernel_spmd

# Problem constants (hardcoded per harness contract).
N, L, H, Q = 64, 258, 1024, 256
WINDOW = 16.0
DEV_POW = 128.0
NCORES = 8
B = N // NCORES  # batches per core
W = 33           # window width (positions that can survive the mask)
HC = H // 128    # h-chunks of 128 (PE contraction tiles)
PK = H + B * W   # packed [W_aT | enc_wT] row width
F32 = mybir.dt.float32
F32R = mybir.dt.float32r
F16 = mybir.dt.float16
AF = mybir.ActivationFunctionType

# exp is computed as t = exp(s/4 + bias); bias = LOG_ALPHA keeps the
# column-sum T = sum_j t below fp32 max.  alpha cancels in w = t/T.
LOG_ALPHA = -4.8520302  # -7*ln(2)
MASK_BIAS = -10000.0    # exp(<= -9900) == 0 in fp32

# v-phase window groups: batches gi*3+off live at partitions off*W..off*W+32
GROUPS = [(0, 99), (99, 99), (198, 66)]


def build_nc() -> bass.Bass:
    nc = bacc.Bacc()
    pk16 = nc.declare_dram_parameter("pk16", [H, PK], F16, isOutput=False)
    dec_hT = nc.declare_dram_parameter("dec_hT", [H, B * Q], F16, isOutput=False)
    W_c1T = nc.declare_dram_parameter("W_c1T", [H, H], F16, isOutput=False)
    W_c2T = nc.declare_dram_parameter("W_c2T", [H, H], F16, isOutput=False)
    constsD = nc.declare_dram_parameter("constsD", [3 * W, B + 3], F32, isOutput=False)
    onesD = nc.declare_dram_parameter("onesD", [W, W], F32R, isOutput=False)
    zerosD = nc.declare_dram_parameter("zerosD", [3 * W, 3 * Q], F32R, isOutput=False)
    out = nc.declare_dram_parameter("out", [B * Q, H], F16, isOutput=True)

    with tile.TileContext(nc) as tc:
        with (
            tc.tile_pool(name="const", bufs=1) as cpool,
            tc.tile_pool(name="pkstg", bufs=8) as pkstg,
            tc.tile_pool(name="wa32", bufs=3) as wa32p,
            tc.tile_pool(name="w16", bufs=4) as w16p,
            tc.tile_pool(name="wc1p", bufs=8) as wc1p,
            tc.tile_pool(name="vstp", bufs=6) as vstp,
            tc.tile_pool(name="dec16p", bufs=2) as dec16p,
            tc.tile_pool(name="dec", bufs=3) as dec_pool,
            tc.tile_pool(name="sm", bufs=4) as sm_pool,
            tc.tile_pool(name="outp", bufs=2) as out_pool,
            tc.tile_pool(name="psA", bufs=2, space="PSUM") as psA,
            tc.tile_pool(name="psB", bufs=6, space="PSUM") as psB,
        ):
            # ---------------- resident tensors ----------------
            enc32 = cpool.tile([128, HC, B * W], F32R)
            Wc2T_sb = cpool.tile([128, HC, H], F32R)
            consts_sb = cpool.tile([3 * W, B + 3], F32)
            ones_sb = cpool.tile([W, W], F32R)
            uT_sb = cpool.tile([128, HC, B * W], F32R)
            # zero-padded align stacks (plane o = batches with n%3 == o):
            # ctx contracts over a whole v group; batch n's 33 softmax rows
            # sit at partition (n%3)*W of plane n%3, zero rows cancel the
            # other batches' windows exactly.
            t3_sb = cpool.tile([3 * W, 3, Q], F32R)
            bias_ap = consts_sb[0:W, 0:B]

            pk_r = pk16[:, :].rearrange("(c p) m -> p c m", p=128)
            Wc2_r = W_c2T[:, :].rearrange("(c p) m -> p c m", p=128)
            Wc1_r = W_c1T[:, :].rearrange("(cp p) m -> p cp m", p=128)
            dec_r = dec_hT[:, :].rearrange("(c p) (n q) -> p c n q", p=128, q=Q)
            zeros_r = zerosD[:, :].rearrange("p (o q) -> p o q", q=Q)

            # ---------------- DMA kickoff (issue order == priority) -------
            # chunk 0 lands in three pieces so the very first u matmul
            # (W_aT cols 0:128 + enc cols) can start ~2us earlier
            # each queue's items are ordered by consumption deadline;
            # per-queue bandwidth is ~105GB/s so balance bytes too
            pk_stage = {}
            st0 = pkstg.tile([128, PK], F16, tag="pk", name="pk0")
            nc.sync.dma_start(out=st0[:, 0:128], in_=pk_r[:, 0, 0:128])
            nc.gpsimd.dma_start(out=st0[:, H:PK], in_=pk_r[:, 0, H:PK])
            nc.scalar.dma_start(out=st0[:, 128:H], in_=pk_r[:, 0, 128:H])
            pk_stage[0] = st0
            qname = {0: nc.sync, 1: nc.gpsimd, 2: nc.scalar}
            pk_q = [None, 0, 1, 2, 0, 1, 2, 0]  # queue per chunk
            for kc in range(1, HC):
                st = pkstg.tile([128, PK], F16, tag="pk", name=f"pk{kc}")
                qname[pk_q[kc]].dma_start(out=st, in_=pk_r[:, kc, :])
                pk_stage[kc] = st
            nc.gpsimd.dma_start(out=consts_sb, in_=constsD[:, :])
            nc.gpsimd.dma_start(out=ones_sb, in_=onesD[:, :])

            # W_c1 fp16 stages, kcp-granular, in v-phase consumption order
            wc1_stage = {}
            wc1_q = {(0, 0): nc.sync, (0, 1): nc.gpsimd, (0, 2): nc.scalar,
                     (0, 3): nc.gpsimd, (1, 0): nc.sync, (1, 1): nc.gpsimd,
                     (1, 2): nc.scalar, (1, 3): nc.gpsimd}
            for nt in range(2):
                for kcp in range(4):
                    st = w16p.tile([128, 2, 512], F16, tag="w16",
                                   name=f"wc1s_{nt}_{kcp}")
                    wc1_q[(nt, kcp)].dma_start(
                        out=st,
                        in_=Wc1_r[:, 2 * kcp:2 * kcp + 2, nt * 512:(nt + 1) * 512],
                    )
                    wc1_stage[(nt, kcp)] = st

            # dec batch 0 (needed ~27us), then W_c2 quarters (~31-37us);
            # t3 zeros before wc2q3 (needed by the first softmax_Z ~34us)
            dec16_tiles = {}
            d16 = dec16p.tile([128, HC, Q], F16, tag="d16", name="d16_0")
            nc.scalar.dma_start(out=d16, in_=dec_r[:, :, 0, :])
            dec16_tiles[0] = d16
            wc2_stage = {}
            wc2_q = [nc.sync, nc.gpsimd, nc.scalar, None]
            for qtr in range(3):
                st = w16p.tile([128, HC, 256], F16, tag="w16", name=f"wc2s_{qtr}")
                wc2_q[qtr].dma_start(out=st, in_=Wc2_r[:, :, qtr * 256:(qtr + 1) * 256])
                wc2_stage[qtr] = st
            nc.sync.dma_start(out=t3_sb[:, :, :], in_=zeros_r)
            st = w16p.tile([128, HC, 256], F16, tag="w16", name="wc2s_3")
            nc.sync.dma_start(out=st, in_=Wc2_r[:, :, 768:1024])
            wc2_stage[3] = st

            # ---------------- u phase: uT[h, (n,j)], kc-outer, f32r -------
            # upcast each chunk as it lands (W_aT part into a rotating
            # buffer, enc part into its keeper), stream through all 8
            # output chunks / PSUM banks.
            pu = {}
            for kc in range(HC):
                wa32 = wa32p.tile([128, H], F32R, tag="wa", name=f"wa32_{kc}")
                if kc == 0:
                    nc.vector.tensor_copy(out=wa32[:, 0:128], in_=pk_stage[0][:, 0:128])
                    nc.vector.tensor_copy(out=enc32[:, 0, :], in_=pk_stage[0][:, H:PK])
                    nc.vector.tensor_copy(out=wa32[:, 128:H], in_=pk_stage[0][:, 128:H])
                else:
                    nc.vector.tensor_copy(out=wa32, in_=pk_stage[kc][:, 0:H])
                    nc.vector.tensor_copy(out=enc32[:, kc, :], in_=pk_stage[kc][:, H:PK])
                for hc in range(HC):
                    if kc == 0:
                        pool = psB if hc < 6 else psA
                        tag = "B" if hc < 6 else "A"
                        pu[hc] = pool.tile([128, B * W], F32, tag=tag, name=f"pu{hc}")
                    nc.tensor.matmul(
                        pu[hc],
                        lhsT=wa32[:, hc * 128:(hc + 1) * 128],
                        rhs=enc32[:, kc, :],
                        start=(kc == 0),
                        stop=(kc == HC - 1),
                    )
            # Upcasts + pu evacuations, interleaved per engine in the order
            # their inputs land / their outputs are consumed (in-order
            # queues: one late op at a queue head blocks everything behind).
            # dec0 lands early on the scalar queue; wc1 chunks land 12-20us;
            # pu tiles only exist once the u phase ends (~20us); the v
            # phase recycles pu PSUM banks in order.
            dec32_tiles = {}

            def dec_cast(n, eng=None):
                d16 = dec16_tiles.pop(n)
                if n + 2 < B and n + 2 not in dec16_tiles:
                    nxt = dec16p.tile([128, HC, Q], F16, tag="d16", name=f"d16_{n+2}")
                    nc.scalar.dma_start(out=nxt, in_=dec_r[:, :, n + 2, :])
                    dec16_tiles[n + 2] = nxt
                dec_sb = dec_pool.tile([128, HC, Q], F32R, tag="dec", name=f"dec{n}")
                (eng or nc.vector).tensor_copy(out=dec_sb, in_=d16)
                dec32_tiles[n] = dec_sb

            def evac(hc):
                if hc % 2 == 0:
                    nc.vector.tensor_copy(out=uT_sb[:, hc, :], in_=pu[hc])
                else:
                    nc.scalar.copy(out=uT_sb[:, hc, :], in_=pu[hc])

            wc1_tiles = {}

            def wc1_cast(nt, kcp):
                wt = wc1p.tile([128, 2, 512], F32R, tag="wc1", name=f"wc1_{nt}_{kcp}")
                if kcp % 2 == 0:
                    nc.scalar.copy(out=wt, in_=wc1_stage[(nt, kcp)])
                else:
                    nc.vector.tensor_copy(out=wt, in_=wc1_stage[(nt, kcp)])
                wc1_tiles[(nt, kcp)] = wt

            dec_cast(0)                      # vector
            wc1_cast(0, 0)                   # scalar
            wc1_cast(0, 1)                   # vector
            wc1_cast(0, 2)                   # scalar
            wc1_cast(0, 3)                   # vector
            evac(0)                          # vector (pv(0,0) bank)
            evac(1)                          # scalar
            wc1_cast(1, 0)                   # scalar
            wc1_cast(1, 1)                   # vector
            wc1_cast(1, 2)                   # scalar
            wc1_cast(1, 3)                   # vector
            for hc in range(2, HC):
                evac(hc)

            # ---------------- v phase: v[(n,j), h'] = enc_w @ W_c1T -------
            # vst tiles stay resident; ctx contracts over the whole group.
            vst_tiles = {}
            for nt in range(2):
                for gi in range(3):
                    g0, glen = GROUPS[gi]
                    pv = psB.tile([128, 512], F32, tag="B", name=f"pv{nt}_{gi}")
                    for kcp in range(4):
                        for j in range(2):
                            kc = 2 * kcp + j
                            nc.tensor.matmul(
                                pv[:glen, :],
                                lhsT=enc32[:, kc, g0:g0 + glen],
                                rhs=wc1_tiles[(nt, kcp)][:, j, :],
                                start=(kc == 0),
                                stop=(kc == HC - 1),
                            )
                    vst = vstp.tile([128, 512], F32R, tag="vst", name=f"vst{nt}_{gi}")
                    # evacuate + fold the gaussian in one op
                    gp = consts_sb[0:glen, B + gi:B + gi + 1]
                    if (nt + gi) % 2 == 0:
                        nc.vector.tensor_scalar_mul(vst[:glen, :], pv[:glen, :], gp)
                    else:
                        nc.scalar.activation(
                            out=vst[:glen, :], in_=pv[:glen, :], func=AF.Copy, scale=gp
                        )
                    vst_tiles[(nt, gi)] = vst

            # W_c2 upcasts in consumption order (q3's happens after the
            # first exp — its stage lands last and must not block exp(0))
            def wc2_cast(qtr):
                dst = Wc2T_sb[:, :, qtr * 256:(qtr + 1) * 256]
                if qtr % 2 == 0:
                    nc.scalar.copy(out=dst, in_=wc2_stage[qtr])
                else:
                    nc.vector.tensor_copy(out=dst, in_=wc2_stage[qtr])

            wc2_cast(0)
            wc2_cast(1)
            wc2_cast(2)

            # ---------------- per-batch pipeline ----------------
            prev = None  # (n, pos, o_sb) awaiting tanh + store

            def flush_prev():
                nonlocal prev
                if prev is None:
                    return
                pn, ppos, po_sb = prev
                for qt in range(2):
                    for ht in range(2):
                        nc.scalar.activation(
                            out=po_sb[:, qt, ht * 512:(ht + 1) * 512],
                            in_=ppos[(qt, ht)], func=AF.Tanh,
                        )
                dst = out[pn * Q:(pn + 1) * Q, :].rearrange("(qt p) h -> p qt h", p=128)
                eng = nc.sync if pn % 2 == 0 else nc.gpsimd
                eng.dma_start(out=dst, in_=po_sb)
                prev = None

            state = {}
            scored = {}

            def score_part(n):
                if n not in dec32_tiles:
                    dec_cast(n)
                dec_sb = dec32_tiles.pop(n)
                ps = psA.tile([W, Q], F32, tag="A", name=f"ps{n}")
                for hc in range(HC):
                    nc.tensor.matmul(
                        ps,
                        lhsT=uT_sb[:, hc, n * W:(n + 1) * W],
                        rhs=dec_sb[:, hc, :],
                        start=(hc == 0),
                        stop=(hc == HC - 1),
                    )
                # softmax over 33 partitions via 4th-power renormalization:
                # t=exp(s/4+b); T=colsum t; t=(t/T)^4; Z=colsum t; t/=Z.
                t = sm_pool.tile([W, Q], F32R, tag="t", name=f"t{n}")
                nc.scalar.activation(
                    out=t, in_=ps, func=AF.Exp, bias=bias_ap[:, n:n + 1], scale=0.25
                )
                scored[n] = (dec_sb, t)

            def softmax_T(n, t):
                pT = psA.tile([W, Q], F32, tag="A", name=f"pT{n}")
                nc.tensor.matmul(pT, lhsT=ones_sb[:], rhs=t[:], start=True, stop=True)
                rT = sm_pool.tile([W, Q], F32, tag="rT", name=f"rT{n}")
                nc.vector.reciprocal_approx_fast(out=rT, in_=pT)
                nc.vector.tensor_mul(t, t, rT)
                nc.vector.tensor_mul(t, t, t)
                nc.vector.tensor_mul(t, t, t)

            def softmax_Z(n, t):
                pZ = psA.tile([W, Q], F32, tag="A", name=f"pZ{n}")
                nc.tensor.matmul(pZ, lhsT=ones_sb[:], rhs=t[:], start=True, stop=True)
                rZ = sm_pool.tile([W, Q], F32, tag="rZ", name=f"rZ{n}")
                nc.vector.reciprocal_approx_fast(out=rZ, in_=pZ)
                tf = sm_pool.tile([W, Q], F32R, tag="tf", name=f"tf{n}")
                nc.vector.tensor_mul(tf, t, rZ)
                off = n % 3
                eng = nc.sync if n % 2 == 0 else nc.gpsimd
                eng.dma_start(out=t3_sb[off * W:(off + 1) * W, off, :], in_=tf[:, :])

            def dec_group(n, dec_sb, pos, qt, ht, start=True, stop=False):
                if start:
                    po = psB.tile([128, 512], F32, tag="B", name=f"po{n}_{qt}_{ht}")
                    pos[(qt, ht)] = po
                else:
                    po = pos[(qt, ht)]
                for hc in range(HC):
                    nc.tensor.matmul(
                        po,
                        lhsT=dec_sb[:, hc, qt * 128:(qt + 1) * 128],
                        rhs=Wc2T_sb[:, hc, ht * 512:(ht + 1) * 512],
                        start=start and (hc == 0),
                        stop=stop and (hc == HC - 1),
                    )

            def ctx_mm(n, pos, qt, ht, stop=True):
                gi, off = divmod(n, 3)
                glen = GROUPS[gi][1]
                nc.tensor.matmul(
                    pos[(qt, ht)],
                    lhsT=t3_sb[0:glen, off, qt * 128:(qt + 1) * 128],
                    rhs=vst_tiles[(ht, gi)][0:glen, :],
                    start=False,
                    stop=stop,
                )

            def batch_pre(n):
                if n not in scored:
                    score_part(n)
                dec_sb, t = scored.pop(n)
                flush_prev()
                o_sb = out_pool.tile([128, 2, H], F16, tag="o", name=f"o{n}")
                pos = {}
                # ht=0 groups first: W_c2's upper half may still be landing
                dec_group(n, dec_sb, pos, 0, 0)
                softmax_T(n, t)
                dec_group(n, dec_sb, pos, 1, 0)
                dec_group(n, dec_sb, pos, 0, 1)
                softmax_Z(n, t)
                dec_group(n, dec_sb, pos, 1, 1)
                state[n] = (pos, o_sb)

            def batch_ctx(n):
                pos, o_sb = state.pop(n)
                for qt in range(2):
                    for ht in range(2):
                        ctx_mm(n, pos, qt, ht)
                nonlocal prev
                prev = (n, pos, o_sb)

            def batch_last(n):
                # softmax for n already ran during batch n-1; each W_c2
                # quarter closes with its ctx and drains tanh+store while
                # the next quarter streams.
                dec_sb, _ = scored.pop(n)
                flush_prev()
                o_sb = out_pool.tile([128, 2, H], F16, tag="o", name=f"o{n}")
                dst = out[n * Q:(n + 1) * Q, :].rearrange("(qt p) h -> p qt h", p=128)
                pos = {}

                def drain(qt, ht):
                    ctx_mm(n, pos, qt, ht)
                    nc.scalar.activation(
                        out=o_sb[:, qt, ht * 512:(ht + 1) * 512],
                        in_=pos[(qt, ht)], func=AF.Tanh,
                    )
                    eng = nc.sync if (qt + ht) % 2 == 0 else nc.gpsimd
                    eng.dma_start(
                        out=dst[:, qt, ht * 512:(ht + 1) * 512],
                        in_=o_sb[:, qt, ht * 512:(ht + 1) * 512],
                    )

                dec_group(n, dec_sb, pos, 0, 0)
                dec_group(n, dec_sb, pos, 0, 1)
                drain(0, 0)
                drain(0, 1)
                dec_group(n, dec_sb, pos, 1, 0)
                drain(1, 0)
                dec_group(n, dec_sb, pos, 1, 1)
                drain(1, 1)

            score_part(0)
            # q3 of W_c2 lands last; its upcast and the dec1 fetch go here
            # so they can't block exp(0) on the scalar queue
            wc2_cast(3)
            d16 = dec16p.tile([128, HC, Q], F16, tag="d16", name="d16_1")
            nc.scalar.dma_start(out=d16, in_=dec_r[:, :, 1, :])
            dec16_tiles[1] = d16
            for n in range(B - 1):
                batch_pre(n)
                if n == B - 2:
                    # emit the last batch's score+softmax here so its t3 is
                    # ready before batch_last's interleaved ctx/drain
                    score_part(B - 1)
                    _, t_last = scored[B - 1]
                    softmax_T(B - 1, t_last)
                    softmax_Z(B - 1, t_last)
                batch_ctx(n)
            batch_last(B - 1)
    nc.compile()
    return nc


def prepare_in_maps(inputs: dict) -> list[dict]:
    enc = np.asarray(inputs["encoder_outputs"], dtype=np.float32)
    dec = np.asarray(inputs["decoder_h_t"], dtype=np.float32)
    src_len = np.asarray(inputs["src_len"], dtype=np.int32)
    p_t = np.asarray(inputs["p_t"], dtype=np.float32)
    W_a = np.asarray(inputs["W_a"], dtype=np.float32)
    W_c = np.asarray(inputs["W_c"], dtype=np.float32)

    # Window bounds, computed with the same fp32 ops as the reference.
    attn_start = np.maximum(p_t - np.float32(WINDOW), np.float32(0.0))
    attn_end = np.minimum(p_t + np.float32(WINDOW), src_len.astype(np.float32))
    s = np.ceil(attn_start).astype(np.int64)
    s = np.minimum(s, L - W)  # keep the 33-slice in bounds
    idx = s[:, None] + np.arange(W)[None, :]
    idxf = idx.astype(np.float32)
    mask = (idxf < attn_start[:, None]) | (idxf > attn_end[:, None])
    bias = np.where(mask, np.float32(MASK_BIAS), np.float32(LOG_ALPHA)).astype(np.float32)
    g = np.exp(-((idxf - p_t[:, None]) ** 2) / np.float32(DEV_POW)).astype(np.float32)

    enc_w = enc[np.arange(N)[:, None], idx, :].astype(np.float16)  # [N, W, H]
    dec = dec.astype(np.float16)
    W_aT16 = W_a.T.astype(np.float16)
    W_c1T = W_c[:, :H].T.astype(np.float16)
    W_c2T = W_c[:, H:].T.astype(np.float16)

    in_maps = []
    for c in range(NCORES):
        bs = slice(c * B, (c + 1) * B)
        enc_wT = enc_w[bs].transpose(2, 0, 1).reshape(H, B * W)  # [H, B*W]
        pk = np.concatenate([W_aT16, enc_wT], axis=1)            # [H, H+B*W]
        gc = g[bs]  # [B, W]
        consts = np.zeros((3 * W, B + 3), dtype=np.float32)
        consts[0:W, 0:B] = bias[bs].T
        for n in range(B):
            gi, off = divmod(n, 3)
            consts[off * W:(off + 1) * W, B + gi] = gc[n]
        in_maps.append({
            "pk16": np.ascontiguousarray(pk),
            "dec_hT": np.ascontiguousarray(dec[bs].transpose(2, 0, 1).reshape(H, B * Q)),
            "W_c1T": W_c1T,
            "W_c2T": W_c2T,
            "constsD": consts,
            "onesD": np.ones((W, W), dtype=np.float32),
            "zerosD": np.zeros((3 * W, 3 * Q), dtype=np.float32),
        })
    return in_maps


_NC = None


def get_nc() -> bass.Bass:
    global _NC
    if _NC is None:
        _NC = build_nc()
    return _NC


def kernel(**inputs) -> np.ndarray:
    nc = get_nc()
    in_maps = prepare_in_maps(inputs)
    res = run_bass_kernel_spmd(nc, in_maps, list(range(NCORES)))
    outs = [np.asarray(res.results[c]["out"], dtype=np.float32).reshape(B, Q, H)
            for c in range(NCORES)]
    return np.concatenate(outs, axis=0)
